# revision 62
# baseline (speedup 1.0000x reference)
"""CRF negative-log-likelihood loss kernel for Trainium2 (Bass/Tile).

Segmented-forward-scan strategy (data-parallel over batch, 8 cores x 32 rows):

  The CRF forward recursion a_t = exp(x_t - c) * (E^T a_{t-1}) is a product
  of strictly positive matrices, so it contracts the Hilbert projective
  metric by ~tanh(diam(E)/4) < 0.5 per step: the scan direction forgets its
  start in ~12 steps.  The T=512 serial scan therefore splits into S=16
  INDEPENDENT segments, each started from ones with a BURN=12 step burn-in.
  All 16 segment chains advance in lockstep as 2 merged groups of 8, so one
  round = 2 matmuls [128x128]@[128x256] + 2 elementwise multiplies -- the
  serial depth drops from 511 steps to 44 rounds.

  Scale stitching (exact, per row): chain s's value v_s is parallel to the
  true alpha, off by a per-row scalar.  With sums N_s = 1.v_s(t_s) (round 44)
  and D_s = 1.v_s(t_{s-1}) (round BURN), ln rho_s = ln N_s - ln D_{s+1}
  telescopes the scales:
      logZ_b = ln(1.v_{s*}(L_b-1)) + sum_{s<s*} ln rho_s + c*K_b,
  with s* the segment owning time L_b-1 and K_b a host-known step count.
  Chain 1 starts exactly from exp(x_0 - c), so no boundary-0 correction.

  Gold score: unary values are host-rebucketed by owning partition into a
  tiny raw region (pure layout gather of the input logits) and summed on
  device; pair score is <C, trans> with C the host-built transition-count
  matrix.  Per-core partial losses summed on host.
"""

import numpy as np

B, T, K = 256, 512, 128
NCORES = 8
BL = B // NCORES          # 32 batch rows per core
S = 24                    # independent segments
BURN = 8                  # burn-in rounds (direction converges ~0.46^BURN)
SEG = 21                  # real steps per segment (chains 2..S)
R = BURN + SEG            # compute rounds per chain (chain 1: t=1..29 real)
NRB = R + 1               # round blocks incl. init
BLK = S * BL              # 768 cols per round block
CT = NRB * BLK            # 23040 exe/hist cols
C_LOG = 5.9               # fixed per-step log rescale (exp bias)
NU2 = 128                 # unary slots per partition (max seen 93)
TB = [R + SEG * (s - 1) for s in range(1, S + 1)]  # t_s boundaries

_CACHE = {}


def _build_program():
    from contextlib import ExitStack

    import concourse.bass as bass
    import concourse.mybir as mybir
    import concourse.tile as tile
    from concourse import bacc

    f32 = mybir.dt.float32
    bf16 = mybir.dt.bfloat16
    i16 = mybir.dt.int16
    AX = mybir.AxisListType
    OP = mybir.AluOpType
    ACTF = mybir.ActivationFunctionType

    nc = bacc.Bacc("TRN2", target_bir_lowering=False, debug=False)

    raw_d = nc.dram_tensor("raw_all", [128, CT], bf16, kind="ExternalInput").ap()
    ureg_d = nc.dram_tensor("ureg", [128, NU2], bf16, kind="ExternalInput").ap()
    trans_d = nc.dram_tensor("trans", [K, K], f32, kind="ExternalInput").ap()
    cmat_d = nc.dram_tensor("cmat", [K, K], f32, kind="ExternalInput").ap()
    krow_d = nc.dram_tensor("krow", [1, BL], f32, kind="ExternalInput").ap()
    mrow_d = nc.dram_tensor("mrow", [1, (S - 1) * BL], f32, kind="ExternalInput").ap()
    idxcap_d = nc.dram_tensor("idx_cap", [128, 2], i16, kind="ExternalInput").ap()
    loss_d = nc.dram_tensor("loss", [1, 1], f32, kind="ExternalOutput").ap()

    # dma/exp chunk boundaries: one small first chunk for a fast start,
    # then 2-round-block chunks
    CHB = [0, BLK, 2 * BLK] + [2 * BLK * k for k in range(2, NRB // 2 + 1)]
    if CHB[-1] != CT:
        CHB.append(CT)
    NCH = len(CHB) - 1

    with tile.TileContext(nc) as tc, ExitStack() as ctx:
        big_pool = ctx.enter_context(tc.tile_pool(name="big", bufs=1))
        small_pool = ctx.enter_context(tc.tile_pool(name="small", bufs=1))
        ps_a = ctx.enter_context(tc.tile_pool(name="psa", bufs=2, space="PSUM"))
        ps_b = ctx.enter_context(tc.tile_pool(name="psb", bufs=2, space="PSUM"))
        ps_misc = ctx.enter_context(tc.tile_pool(name="ps_misc", bufs=1, space="PSUM"))

        exe = big_pool.tile([128, CT], bf16, tag="exe")
        hist = big_pool.tile([128, CT], bf16, tag="hist")

        trs = small_pool.tile([K, K], f32, tag="trs")
        cmat = small_pool.tile([K, K], f32, tag="cmat")
        e_bf = small_pool.tile([K, K], bf16, tag="e_bf")
        ureg = small_pool.tile([128, NU2], bf16, tag="ureg")
        krow = small_pool.tile([1, BL], f32, tag="krow")
        mrow = small_pool.tile([1, (S - 1) * BL], f32, tag="mrow")
        idx_cap = small_pool.tile([128, 2], i16, tag="idx_cap")
        bias_c = small_pool.tile([128, 1], f32, tag="bias_c")
        ones_bf = small_pool.tile([128, 1], bf16, tag="ones_bf")
        ones_f = small_pool.tile([128, 1], f32, tag="ones_f")

        u_junk = small_pool.tile([128, NU2], f32, tag="u_junk")
        u_acc = small_pool.tile([128, 1], f32, tag="u_acc")
        pair_junk = small_pool.tile([128, K], f32, tag="pair_junk")
        pair_acc = small_pool.tile([128, 1], f32, tag="pair_acc")
        ga = small_pool.tile([128, 64], bf16, tag="ga")
        lnn = small_pool.tile([1, BLK], f32, tag="lnn")
        lnd = small_pool.tile([1, BLK], f32, tag="lnd")
        lnr = small_pool.tile([1, (S - 1) * BL], f32, tag="lnr")
        msum = small_pool.tile([1, BL], f32, tag="msum")
        caprow = small_pool.tile([1, BL], f32, tag="caprow")
        lncap = small_pool.tile([1, BL], f32, tag="lncap")
        lzrow = small_pool.tile([1, BL], f32, tag="lzrow")
        t1 = small_pool.tile([1, 1], f32, tag="t1")
        score_tot = small_pool.tile([1, 1], f32, tag="score_tot")
        loss_sb = small_pool.tile([1, 1], f32, tag="loss_sb")

        # ---------------- prologue ----------------
        nc.sync.dma_start(trs[:], trans_d[:, :])

        def dma_chunk(k):
            nc.sync.dma_start(
                exe[:, CHB[k] : CHB[k + 1]], raw_d[:, CHB[k] : CHB[k + 1]]
            )

        def exp_chunk(k):
            nc.scalar.activation(
                exe[:, CHB[k] : CHB[k + 1]], exe[:, CHB[k] : CHB[k + 1]],
                ACTF.Exp, bias=bias_c[:],
            )

        nc.vector.memset(bias_c[:], -C_LOG)
        nc.vector.memset(ones_bf[:], 1.0)
        nc.vector.memset(ones_f[:], 1.0)

        # input-free dummy Exp: loads the scalar ACT table while DMAs run
        warm = small_pool.tile([128, 1], f32, tag="warm")
        nc.scalar.activation(warm[:], bias_c[:], ACTF.Exp)

        dma_chunk(0)
        nc.scalar.activation(e_bf[:], trs[:], ACTF.Exp)
        exp_chunk(0)
        dma_chunk(1)
        dma_chunk(2)
        dma_chunk(3)
        dma_chunk(4)
        dma_chunk(5)
        exp_chunk(1)
        exp_chunk(2)
        exp_chunk(3)
        exp_chunk(4)

        # small tables (sync queue, after the first data chunks)
        nc.sync.dma_start(idx_cap[:], idxcap_d[:, :])
        nc.sync.dma_start(ureg[:], ureg_d[:, :])
        nc.sync.dma_start(cmat[:], cmat_d[:, :])
        nc.sync.dma_start(krow[:], krow_d[:, :])
        nc.sync.dma_start(mrow[:], mrow_d[:, :])

        # gpsimd custom-op library preload (capture gather needs it later)
        dum_src = small_pool.tile([128, 4], bf16, tag="dum_src")
        dum_idx = small_pool.tile([128, 1], i16, tag="dum_idx")
        dum_out = small_pool.tile([128, 32], bf16, tag="dum_out")
        nc.gpsimd.memset(dum_src[:], 0.0)
        nc.gpsimd.memset(dum_idx[:], 0)
        nc.gpsimd.ap_gather(
            dum_out[:], dum_src[:], dum_idx[:], channels=128,
            num_elems=2, d=2, num_idxs=16,
        )

        # init: hist round-0 block = exe round-0 block (host: chain1=exp(x0-c),
        # others raw 0 -> exp -> ones)
        nc.vector.tensor_copy(hist[:, 0:BLK], exe[:, 0:BLK])

        # ---------------- the scan: 44 rounds x 2 merged groups ----------------
        HB = BLK // 2  # cols per group
        for r in range(1, R + 1):
            # chunk k (k>=2) covers rounds 2(k-1), 2(k-1)+1; stay ~4 ahead
            if r % 2 == 0:
                k = r // 2 + 5
                if k < NCH:
                    dma_chunk(k)
            if r % 2 == 1:
                k = (r + 9) // 2
                if k < NCH:
                    exp_chunk(k)

            for g, pool in ((0, ps_a), (1, ps_b)):
                lo = (r - 1) * BLK + g * HB
                oo = r * BLK + g * HB
                up = pool.tile([K, HB], f32, tag=f"up{g}")
                nc.tensor.matmul(up[:], e_bf[:], hist[:, lo : lo + HB], start=True, stop=True)
                nc.vector.tensor_mul(hist[:, oo : oo + HB], up[:], exe[:, oo : oo + HB])

        # ---------------- epilogue (low priority: keep off scan queues) ------
        ctx.enter_context(tc.high_priority(offset=-(10**6)))

        # gold score: unary region sum + <C, trans>, both off the DVE
        nc.scalar.activation(u_junk[:], ureg[:], ACTF.Copy, accum_out=u_acc[:])
        nc.vector.scalar_tensor_tensor(
            pair_junk[:], cmat[:], 1.0, trs[:], OP.mult, OP.mult,
            accum_out=pair_acc[:],
        )
        mi_ps = ps_misc.tile([1, 34], f32, tag="mm_misc")
        sc_ps = mi_ps[:, 32:33]
        nc.tensor.matmul(sc_ps, ones_f[:], u_acc[:], start=True, stop=False)
        nc.tensor.matmul(sc_ps, ones_f[:], pair_acc[:], start=False, stop=True)
        nc.vector.tensor_copy(score_tot[:], sc_ps)

        # boundary sums: N over the last round block, D over the round-BURN
        # block; [1, BLK] exceeds a PSUM bank, so two halves, tiles reused
        # den -> num.
        HBK = BLK // 2
        bnd = [
            ps_misc.tile([1, HBK], f32, tag=f"mm_bnd{h}", name=f"bnd{h}")
            for h in range(2)
        ]
        for h in range(2):
            nc.tensor.matmul(
                bnd[h][:], ones_bf[:],
                hist[:, BURN * BLK + h * HBK : BURN * BLK + (h + 1) * HBK],
                start=True, stop=True,
            )
            nc.scalar.activation(lnd[:, h * HBK : (h + 1) * HBK], bnd[h][:], ACTF.Ln)
        for h in range(2):
            nc.tensor.matmul(
                bnd[h][:], ones_bf[:],
                hist[:, R * BLK + h * HBK : R * BLK + (h + 1) * HBK],
                start=True, stop=True,
            )
            nc.scalar.activation(lnn[:, h * HBK : (h + 1) * HBK], bnd[h][:], ACTF.Ln)
        # ln rho_s[b] = ln N_s - ln D_{s+1}, masked per row then summed over s
        nc.vector.tensor_sub(lnr[:], lnn[:, 0 : (S - 1) * BL], lnd[:, BL:BLK])
        nc.vector.tensor_tensor(lnr[:], lnr[:], mrow[:], OP.mult)
        nc.vector.tensor_reduce(
            msum[:], lnr[:].rearrange("p (s b) -> p b s", b=BL), AX.X, OP.add
        )

        # capture logZ numerators at per-row (s*, r*) columns
        nc.gpsimd.ap_gather(
            ga[:], hist[:], idx_cap[:, :], channels=128,
            num_elems=CT // 2, d=2, num_idxs=32,
        )
        nc.tensor.matmul(mi_ps[:, 0:16], ones_bf[:], ga[:, 0:64:4], start=True, stop=True)
        nc.tensor.matmul(mi_ps[:, 16:32], ones_bf[:], ga[:, 3:64:4], start=True, stop=True)
        nc.vector.tensor_copy(caprow[:, 0:BL:2], mi_ps[:, 0:16])
        nc.vector.tensor_copy(caprow[:, 1:BL:2], mi_ps[:, 16:32])
        nc.scalar.activation(lncap[:], caprow[:], ACTF.Ln)

        # logZ row = lncap + msum + c*K  (K also folds -L_b from the unary c shift)
        nc.vector.tensor_tensor(lzrow[:], lncap[:], msum[:], OP.add)
        nc.vector.scalar_tensor_tensor(
            lzrow[:], krow[:], C_LOG, lzrow[:], OP.mult, OP.add
        )
        nc.vector.tensor_reduce(t1[:], lzrow[:], AX.X, OP.add)
        nc.vector.tensor_sub(loss_sb[:], t1[:], score_tot[:])
        nc.sync.dma_start(loss_d[:, :], loss_sb[:])

    nc.compile()
    return nc


def _get_program():
    if "prog" not in _CACHE:
        _CACHE["prog"] = _build_program()
    return _CACHE["prog"]


def _core_tables(lgT_bf, lab, L):
    """Per-core tables. lgT_bf: [K,T,BL] bf16, lab: [BL,T], L: [BL]."""
    import ml_dtypes

    bf = ml_dtypes.bfloat16
    t = {}
    # raw exe table [k, r, s, b]: chain 1 covers t=r (r=0 is the exact init);
    # chains s>=2 start from ones at t_{s-1}-BURN (raw 0 -> exp -> 1).
    tbm1 = np.array([0] + TB)  # tbm1[s] = t_{s-1} boundary for chain s (1-based)
    tidx = np.zeros((NRB, S), np.int64)
    tidx[:, 0] = np.arange(NRB)
    for s in range(2, S + 1):
        tidx[:, s - 1] = tbm1[s - 1] - BURN + np.arange(NRB)
    tidx = np.clip(tidx, 0, T - 1)
    raw = lgT_bf[:, tidx, :]              # [K, NRB, S, BL]
    raw[:, 0, 1:, :] = np.float32(0.0)    # ones-init for chains >= 2
    t["raw_all"] = np.ascontiguousarray(raw.reshape(128, CT), dtype=bf)

    # unary region: values logits[b,t,lab] bucketed by owning partition k
    bb, tt = np.nonzero(np.arange(T)[None, :] < L[:, None])
    kk = lab[bb, tt]
    vals = lgT_bf[kk, tt, bb].astype(np.float32)
    ureg = np.zeros((128, NU2), np.float32)
    order = np.argsort(kk, kind="stable")
    kk_s, v_s = kk[order], vals[order]
    counts = np.bincount(kk_s, minlength=128)
    assert counts.max() <= NU2, f"unary overflow: {counts.max()}"
    off = 0
    for p in range(128):
        n = counts[p]
        ureg[p, :n] = v_s[off : off + n]
        off += n
    t["ureg"] = ureg.astype(bf)

    # pair count matrix
    act = (np.arange(T - 1)[None, :] + 1) < L[:, None]
    cmat = np.zeros((K, K), np.float32)
    np.add.at(cmat, (lab[:, :-1][act], lab[:, 1:][act]), 1.0)
    t["cmat"] = cmat

    # capture indices + stitch masks + c-exponent row
    s_star = np.searchsorted(np.array(TB), L - 1) + 1       # [BL], 1..S
    r_star = np.where(s_star == 1, L - 1, L - 1 - tbm1[s_star - 1] + BURN)
    cap_col = r_star * BLK + (s_star - 1) * BL + np.arange(BL)
    p = np.arange(128)[:, None]
    cgrid = np.arange(2)[None, :]
    bcap = cgrid * 16 + (p % 16)
    del cap_col
    t["idx_cap"] = (
        (r_star[bcap] * BLK + (s_star[bcap] - 1) * BL + bcap) // 2
    ).astype(np.int16)

    K_b = np.where(
        s_star == 1,
        L.astype(np.int64),
        (L - 1 - tbm1[s_star - 1] + BURN) + (SEG + 1) + SEG * (s_star - 2),
    )
    # fold the unary ln-shift: ureg holds raw x (no -c), so no shift needed here;
    # krow carries c*K_b only.
    t["krow"] = K_b.astype(np.float32).reshape(1, BL)
    # mrow[s-1, b] = 1 if boundary s is before row b's capture segment (s < s*)
    sgrid = np.arange(1, S)[:, None]
    t["mrow"] = (sgrid < s_star[None, :]).astype(np.float32).reshape(1, (S - 1) * BL)
    return t


def _make_in_maps(logits, labels, seq_lens, trans):
    import ml_dtypes

    bf = ml_dtypes.bfloat16
    logits = np.asarray(logits, dtype=np.float32)
    labels = np.asarray(labels, dtype=np.int64)
    seq_lens = np.asarray(seq_lens, dtype=np.int64)
    trans = np.asarray(trans, dtype=np.float32)

    in_maps = []
    for c in range(NCORES):
        sl = slice(c * BL, (c + 1) * BL)
        lgT_bf = logits[sl].transpose(2, 1, 0).astype(bf)  # [K, T, BL]
        m = {"trans": trans}
        m.update(_core_tables(lgT_bf, labels[sl], seq_lens[sl]))
        in_maps.append(m)
    return in_maps


def kernel(logits, labels, seq_lens, trans):
    from concourse.bass_utils import run_bass_kernel_spmd

    nc = _get_program()
    in_maps = _make_in_maps(logits, labels, seq_lens, trans)
    res = run_bass_kernel_spmd(nc, in_maps, list(range(NCORES)))
    total = sum(float(res.results[c]["loss"][0, 0]) for c in range(NCORES))
    return np.float32(total)


# revision 65
# speedup vs baseline: 1.0058x; 1.0058x over previous
"""CRF negative-log-likelihood loss kernel for Trainium2 (Bass/Tile).

Segmented-forward-scan strategy (data-parallel over batch, 8 cores x 32 rows):

  The CRF forward recursion a_t = exp(x_t - c) * (E^T a_{t-1}) is a product
  of strictly positive matrices, so it contracts the Hilbert projective
  metric by ~tanh(diam(E)/4) < 0.5 per step: the scan direction forgets its
  start in ~12 steps.  The T=512 serial scan therefore splits into S=16
  INDEPENDENT segments, each started from ones with a BURN=12 step burn-in.
  All 16 segment chains advance in lockstep as 2 merged groups of 8, so one
  round = 2 matmuls [128x128]@[128x256] + 2 elementwise multiplies -- the
  serial depth drops from 511 steps to 44 rounds.

  Scale stitching (exact, per row): chain s's value v_s is parallel to the
  true alpha, off by a per-row scalar.  With sums N_s = 1.v_s(t_s) (round 44)
  and D_s = 1.v_s(t_{s-1}) (round BURN), ln rho_s = ln N_s - ln D_{s+1}
  telescopes the scales:
      logZ_b = ln(1.v_{s*}(L_b-1)) + sum_{s<s*} ln rho_s + c*K_b,
  with s* the segment owning time L_b-1 and K_b a host-known step count.
  Chain 1 starts exactly from exp(x_0 - c), so no boundary-0 correction.

  Gold score: unary values are host-rebucketed by owning partition into a
  tiny raw region (pure layout gather of the input logits) and summed on
  device; pair score is <C, trans> with C the host-built transition-count
  matrix.  Per-core partial losses summed on host.
"""

import numpy as np

B, T, K = 256, 512, 128
NCORES = 8
BL = B // NCORES          # 32 batch rows per core
S = 24                    # independent segments
BURN = 6                  # burn-in rounds (direction converges ~0.46^BURN)
SEG = 22                  # real steps per segment (chains 2..S)
R = BURN + SEG            # compute rounds per chain (chain 1: t=1..29 real)
NRB = R + 1               # round blocks incl. init
BLK = S * BL              # 768 cols per round block
CT = NRB * BLK            # 23040 exe/hist cols
C_LOG = 5.9               # fixed per-step log rescale (exp bias)
NU2 = 128                 # unary slots per partition (max seen 93)
TB = [R + SEG * (s - 1) for s in range(1, S + 1)]  # t_s boundaries

_CACHE = {}


def _build_program():
    from contextlib import ExitStack

    import concourse.bass as bass
    import concourse.mybir as mybir
    import concourse.tile as tile
    from concourse import bacc

    f32 = mybir.dt.float32
    bf16 = mybir.dt.bfloat16
    i16 = mybir.dt.int16
    AX = mybir.AxisListType
    OP = mybir.AluOpType
    ACTF = mybir.ActivationFunctionType

    nc = bacc.Bacc("TRN2", target_bir_lowering=False, debug=False)

    raw_d = nc.dram_tensor("raw_all", [128, CT], bf16, kind="ExternalInput").ap()
    ureg_d = nc.dram_tensor("ureg", [128, NU2], bf16, kind="ExternalInput").ap()
    trans_d = nc.dram_tensor("trans", [K, K], f32, kind="ExternalInput").ap()
    cmat_d = nc.dram_tensor("cmat", [K, K], f32, kind="ExternalInput").ap()
    krow_d = nc.dram_tensor("krow", [1, BL], f32, kind="ExternalInput").ap()
    mrow_d = nc.dram_tensor("mrow", [1, (S - 1) * BL], f32, kind="ExternalInput").ap()
    idxcap_d = nc.dram_tensor("idx_cap", [128, 2], i16, kind="ExternalInput").ap()
    loss_d = nc.dram_tensor("loss", [1, 1], f32, kind="ExternalOutput").ap()

    # dma/exp chunk boundaries: one small first chunk for a fast start,
    # then 2-round-block chunks
    CHB = [0, BLK, 2 * BLK] + [2 * BLK * k for k in range(2, NRB // 2 + 1)]
    if CHB[-1] != CT:
        CHB.append(CT)
    NCH = len(CHB) - 1

    with tile.TileContext(nc) as tc, ExitStack() as ctx:
        big_pool = ctx.enter_context(tc.tile_pool(name="big", bufs=1))
        small_pool = ctx.enter_context(tc.tile_pool(name="small", bufs=1))
        ps_a = ctx.enter_context(tc.tile_pool(name="psa", bufs=2, space="PSUM"))
        ps_b = ctx.enter_context(tc.tile_pool(name="psb", bufs=2, space="PSUM"))
        ps_misc = ctx.enter_context(tc.tile_pool(name="ps_misc", bufs=1, space="PSUM"))

        exe = big_pool.tile([128, CT], bf16, tag="exe")
        hist = big_pool.tile([128, CT], bf16, tag="hist")

        trs = small_pool.tile([K, K], f32, tag="trs")
        cmat = small_pool.tile([K, K], f32, tag="cmat")
        e_bf = small_pool.tile([K, K], bf16, tag="e_bf")
        ureg = small_pool.tile([128, NU2], bf16, tag="ureg")
        krow = small_pool.tile([1, BL], f32, tag="krow")
        mrow = small_pool.tile([1, (S - 1) * BL], f32, tag="mrow")
        idx_cap = small_pool.tile([128, 2], i16, tag="idx_cap")
        bias_c = small_pool.tile([128, 1], f32, tag="bias_c")
        ones_bf = small_pool.tile([128, 1], bf16, tag="ones_bf")
        ones_f = small_pool.tile([128, 1], f32, tag="ones_f")

        u_junk = small_pool.tile([128, NU2], f32, tag="u_junk")
        u_acc = small_pool.tile([128, 1], f32, tag="u_acc")
        pair_junk = small_pool.tile([128, K], f32, tag="pair_junk")
        pair_acc = small_pool.tile([128, 1], f32, tag="pair_acc")
        ga = small_pool.tile([128, 64], bf16, tag="ga")
        lnn = small_pool.tile([1, BLK], f32, tag="lnn")
        lnd = small_pool.tile([1, BLK], f32, tag="lnd")
        lnr = small_pool.tile([1, (S - 1) * BL], f32, tag="lnr")
        msum = small_pool.tile([1, BL], f32, tag="msum")
        caprow = small_pool.tile([1, BL], f32, tag="caprow")
        lncap = small_pool.tile([1, BL], f32, tag="lncap")
        lzrow = small_pool.tile([1, BL], f32, tag="lzrow")
        t1 = small_pool.tile([1, 1], f32, tag="t1")
        score_tot = small_pool.tile([1, 1], f32, tag="score_tot")
        loss_sb = small_pool.tile([1, 1], f32, tag="loss_sb")

        # ---------------- prologue ----------------

        def dma_chunk(k):
            nc.sync.dma_start(
                exe[:, CHB[k] : CHB[k + 1]], raw_d[:, CHB[k] : CHB[k + 1]]
            )

        def exp_chunk(k):
            nc.scalar.activation(
                exe[:, CHB[k] : CHB[k + 1]], exe[:, CHB[k] : CHB[k + 1]],
                ACTF.Exp, bias=bias_c[:],
            )

        nc.vector.memset(bias_c[:], -C_LOG)
        nc.vector.memset(ones_bf[:], 1.0)
        nc.vector.memset(ones_f[:], 1.0)

        # input-free dummy Exp: loads the scalar ACT table while DMAs run
        warm = small_pool.tile([128, 1], f32, tag="warm")
        nc.scalar.activation(warm[:], bias_c[:], ACTF.Exp)

        dma_chunk(0)
        nc.sync.dma_start(trs[:], trans_d[:, :])
        nc.scalar.activation(e_bf[:], trs[:], ACTF.Exp)
        exp_chunk(0)
        dma_chunk(1)
        dma_chunk(2)
        dma_chunk(3)
        dma_chunk(4)
        dma_chunk(5)
        exp_chunk(1)
        exp_chunk(2)
        exp_chunk(3)
        exp_chunk(4)

        # small tables (sync queue, after the first data chunks)
        nc.sync.dma_start(idx_cap[:], idxcap_d[:, :])
        nc.sync.dma_start(ureg[:], ureg_d[:, :])
        nc.sync.dma_start(cmat[:], cmat_d[:, :])
        nc.sync.dma_start(krow[:], krow_d[:, :])
        nc.sync.dma_start(mrow[:], mrow_d[:, :])

        # gpsimd custom-op library preload (capture gather needs it later)
        dum_src = small_pool.tile([128, 4], bf16, tag="dum_src")
        dum_idx = small_pool.tile([128, 1], i16, tag="dum_idx")
        dum_out = small_pool.tile([128, 32], bf16, tag="dum_out")
        nc.gpsimd.memset(dum_src[:], 0.0)
        nc.gpsimd.memset(dum_idx[:], 0)
        nc.gpsimd.ap_gather(
            dum_out[:], dum_src[:], dum_idx[:], channels=128,
            num_elems=2, d=2, num_idxs=16,
        )

        # init: hist round-0 block = exe round-0 block (host: chain1=exp(x0-c),
        # others raw 0 -> exp -> ones)
        nc.vector.tensor_copy(hist[:, 0:BLK], exe[:, 0:BLK])

        # ---------------- the scan: 44 rounds x 2 merged groups ----------------
        HB = BLK // 2  # cols per group
        for r in range(1, R + 1):
            # chunk k (k>=2) covers rounds 2(k-1), 2(k-1)+1; stay ~4 ahead
            if r % 2 == 0:
                k = r // 2 + 5
                if k < NCH:
                    dma_chunk(k)
            if r % 2 == 1:
                k = (r + 9) // 2
                if k < NCH:
                    exp_chunk(k)

            for g, pool in ((0, ps_a), (1, ps_b)):
                lo = (r - 1) * BLK + g * HB
                oo = r * BLK + g * HB
                up = pool.tile([K, HB], f32, tag=f"up{g}")
                nc.tensor.matmul(up[:], e_bf[:], hist[:, lo : lo + HB], start=True, stop=True)
                nc.vector.tensor_mul(hist[:, oo : oo + HB], up[:], exe[:, oo : oo + HB])

        # ---------------- epilogue (low priority: keep off scan queues) ------
        ctx.enter_context(tc.high_priority(offset=-(10**6)))

        # gold score: unary region sum + <C, trans>, both off the DVE
        nc.scalar.activation(u_junk[:], ureg[:], ACTF.Copy, accum_out=u_acc[:])
        nc.vector.scalar_tensor_tensor(
            pair_junk[:], cmat[:], 1.0, trs[:], OP.mult, OP.mult,
            accum_out=pair_acc[:],
        )
        mi_ps = ps_misc.tile([1, 34], f32, tag="mm_misc")
        sc_ps = mi_ps[:, 32:33]
        nc.tensor.matmul(sc_ps, ones_f[:], u_acc[:], start=True, stop=False)
        nc.tensor.matmul(sc_ps, ones_f[:], pair_acc[:], start=False, stop=True)
        nc.vector.tensor_copy(score_tot[:], sc_ps)

        # boundary sums: N over the last round block, D over the round-BURN
        # block; [1, BLK] exceeds a PSUM bank, so two halves, tiles reused
        # den -> num.
        HBK = BLK // 2
        bnd = [
            ps_misc.tile([1, HBK], f32, tag=f"mm_bnd{h}", name=f"bnd{h}")
            for h in range(2)
        ]
        for h in range(2):
            nc.tensor.matmul(
                bnd[h][:], ones_bf[:],
                hist[:, BURN * BLK + h * HBK : BURN * BLK + (h + 1) * HBK],
                start=True, stop=True,
            )
            nc.scalar.activation(lnd[:, h * HBK : (h + 1) * HBK], bnd[h][:], ACTF.Ln)
        for h in range(2):
            nc.tensor.matmul(
                bnd[h][:], ones_bf[:],
                hist[:, R * BLK + h * HBK : R * BLK + (h + 1) * HBK],
                start=True, stop=True,
            )
            nc.scalar.activation(lnn[:, h * HBK : (h + 1) * HBK], bnd[h][:], ACTF.Ln)
        # ln rho_s[b] = ln N_s - ln D_{s+1}, masked per row then summed over s
        nc.vector.tensor_sub(lnr[:], lnn[:, 0 : (S - 1) * BL], lnd[:, BL:BLK])
        nc.vector.tensor_tensor(lnr[:], lnr[:], mrow[:], OP.mult)
        nc.vector.tensor_reduce(
            msum[:], lnr[:].rearrange("p (s b) -> p b s", b=BL), AX.X, OP.add
        )

        # capture logZ numerators at per-row (s*, r*) columns
        nc.gpsimd.ap_gather(
            ga[:], hist[:], idx_cap[:, :], channels=128,
            num_elems=CT // 2, d=2, num_idxs=32,
        )
        nc.tensor.matmul(mi_ps[:, 0:16], ones_bf[:], ga[:, 0:64:4], start=True, stop=True)
        nc.tensor.matmul(mi_ps[:, 16:32], ones_bf[:], ga[:, 3:64:4], start=True, stop=True)
        nc.vector.tensor_copy(caprow[:, 0:BL:2], mi_ps[:, 0:16])
        nc.vector.tensor_copy(caprow[:, 1:BL:2], mi_ps[:, 16:32])
        nc.scalar.activation(lncap[:], caprow[:], ACTF.Ln)

        # logZ row = lncap + msum + c*K  (K also folds -L_b from the unary c shift)
        nc.vector.tensor_tensor(lzrow[:], lncap[:], msum[:], OP.add)
        nc.vector.scalar_tensor_tensor(
            lzrow[:], krow[:], C_LOG, lzrow[:], OP.mult, OP.add
        )
        nc.vector.tensor_reduce(t1[:], lzrow[:], AX.X, OP.add)
        nc.vector.tensor_sub(loss_sb[:], t1[:], score_tot[:])
        nc.sync.dma_start(loss_d[:, :], loss_sb[:])

    nc.compile()
    return nc


def _get_program():
    if "prog" not in _CACHE:
        _CACHE["prog"] = _build_program()
    return _CACHE["prog"]


def _core_tables(lgT_bf, lab, L):
    """Per-core tables. lgT_bf: [K,T,BL] bf16, lab: [BL,T], L: [BL]."""
    import ml_dtypes

    bf = ml_dtypes.bfloat16
    t = {}
    # raw exe table [k, r, s, b]: chain 1 covers t=r (r=0 is the exact init);
    # chains s>=2 start from ones at t_{s-1}-BURN (raw 0 -> exp -> 1).
    tbm1 = np.array([0] + TB)  # tbm1[s] = t_{s-1} boundary for chain s (1-based)
    tidx = np.zeros((NRB, S), np.int64)
    tidx[:, 0] = np.arange(NRB)
    for s in range(2, S + 1):
        tidx[:, s - 1] = tbm1[s - 1] - BURN + np.arange(NRB)
    tidx = np.clip(tidx, 0, T - 1)
    raw = lgT_bf[:, tidx, :]              # [K, NRB, S, BL]
    raw[:, 0, 1:, :] = np.float32(0.0)    # ones-init for chains >= 2
    t["raw_all"] = np.ascontiguousarray(raw.reshape(128, CT), dtype=bf)

    # unary region: values logits[b,t,lab] bucketed by owning partition k
    bb, tt = np.nonzero(np.arange(T)[None, :] < L[:, None])
    kk = lab[bb, tt]
    vals = lgT_bf[kk, tt, bb].astype(np.float32)
    ureg = np.zeros((128, NU2), np.float32)
    order = np.argsort(kk, kind="stable")
    kk_s, v_s = kk[order], vals[order]
    counts = np.bincount(kk_s, minlength=128)
    assert counts.max() <= NU2, f"unary overflow: {counts.max()}"
    off = 0
    for p in range(128):
        n = counts[p]
        ureg[p, :n] = v_s[off : off + n]
        off += n
    t["ureg"] = ureg.astype(bf)

    # pair count matrix
    act = (np.arange(T - 1)[None, :] + 1) < L[:, None]
    cmat = np.zeros((K, K), np.float32)
    np.add.at(cmat, (lab[:, :-1][act], lab[:, 1:][act]), 1.0)
    t["cmat"] = cmat

    # capture indices + stitch masks + c-exponent row
    s_star = np.searchsorted(np.array(TB), L - 1) + 1       # [BL], 1..S
    r_star = np.where(s_star == 1, L - 1, L - 1 - tbm1[s_star - 1] + BURN)
    cap_col = r_star * BLK + (s_star - 1) * BL + np.arange(BL)
    p = np.arange(128)[:, None]
    cgrid = np.arange(2)[None, :]
    bcap = cgrid * 16 + (p % 16)
    del cap_col
    t["idx_cap"] = (
        (r_star[bcap] * BLK + (s_star[bcap] - 1) * BL + bcap) // 2
    ).astype(np.int16)

    K_b = np.where(
        s_star == 1,
        L.astype(np.int64),
        (L - 1 - tbm1[s_star - 1] + BURN) + (SEG + 1) + SEG * (s_star - 2),
    )
    # fold the unary ln-shift: ureg holds raw x (no -c), so no shift needed here;
    # krow carries c*K_b only.
    t["krow"] = K_b.astype(np.float32).reshape(1, BL)
    # mrow[s-1, b] = 1 if boundary s is before row b's capture segment (s < s*)
    sgrid = np.arange(1, S)[:, None]
    t["mrow"] = (sgrid < s_star[None, :]).astype(np.float32).reshape(1, (S - 1) * BL)
    return t


def _make_in_maps(logits, labels, seq_lens, trans):
    import ml_dtypes

    bf = ml_dtypes.bfloat16
    logits = np.asarray(logits, dtype=np.float32)
    labels = np.asarray(labels, dtype=np.int64)
    seq_lens = np.asarray(seq_lens, dtype=np.int64)
    trans = np.asarray(trans, dtype=np.float32)

    in_maps = []
    for c in range(NCORES):
        sl = slice(c * BL, (c + 1) * BL)
        lgT_bf = logits[sl].transpose(2, 1, 0).astype(bf)  # [K, T, BL]
        m = {"trans": trans}
        m.update(_core_tables(lgT_bf, labels[sl], seq_lens[sl]))
        in_maps.append(m)
    return in_maps


def kernel(logits, labels, seq_lens, trans):
    from concourse.bass_utils import run_bass_kernel_spmd

    nc = _get_program()
    in_maps = _make_in_maps(logits, labels, seq_lens, trans)
    res = run_bass_kernel_spmd(nc, in_maps, list(range(NCORES)))
    total = sum(float(res.results[c]["loss"][0, 0]) for c in range(NCORES))
    return np.float32(total)
